# revision 29
# baseline (speedup 1.0000x reference)
"""Trainium2 Bass kernel for nn_BandSplit (grouped band einsum as banded matmul).

The reference computes, per (b, t) row:
    g = gather(x, f_idxes) * mask            # per-band slice of the spectrum
    h = einsum('ki,kio->ko', g, pre_weight) + pre_bias
    y = einsum('ko,koj->kj', h, post_weight) + post_bias
    out = scatter_add(y * mask) / ola_window

Because each band's nonzero bins are a contiguous f-range, the whole pipeline
is linear in x and collapses to ONE banded matrix multiply in the interleaved
linear space  lin = f*4 + c  (bandwidth <= 131 < 132):

    out_lin[l', r] = sum_l A[l, l'] * x_lin[l, r]
    A = sum_k scatter(diag(mask_k) @ W1_k @ W2_k @ diag(mask_k / ola))

A is built on the host from the (small) weight inputs.  x is pre-transposed on
the host into [lin, rows] tiles so the device does only contiguous DMA plus
dense 128x128x512 matmuls (fp32 PSUM accumulation) on 3 block-diagonals.

Sharding: 8 lin-groups of 4 out-tiles (of 128) x full rows, one per core.
Each out-tile j contracts its own tile (d=1) and both neighbours (d=0/d=2).
The group-edge halo paths carry only ~5% of the output energy, so they run
in fp8 (E4M3) at their true support width (<=112 below / <=104 above the
group), saving ~45% of the halo DMA bytes; everything else is fp16 with
fp32 PSUM.  Measured end-to-end error ~8e-3 vs the fp32 reference.

DMA layout is chosen for few, large transfers (the SP sequencer pays ~650ns
per issued DMA): one DMA per x tile / weight blob / out tile.  Matmuls are
ordered (out-tile, diagonal, chunk) so the PE keeps one stationary weight
block for 4 consecutive matmuls.
"""

import numpy as np

# ---- problem constants (hardcoded; harness supplies matching inputs) ----
B, C, T, F = 4, 4, 512, 1025
KB, WMAX = 256, 33
L = F * C                 # 4100 linear positions
NT = (L + 127) // 128     # 33 tiles of 128
LPAD = NT * 128           # 4224
R = B * T                 # 2048 rows (b, t)
NCORES = 8
ND = 3                    # block diagonals
CHUNK = 512               # PSUM bank (fp32) free-dim limit

# The last lin-tile (32) covers only 4 real positions (f-bin 1024); its
# output is computed on the host, so the device grid is exactly 32 tiles.
NT_DEV = 32
RES_LO = NT_DEV * 128            # 4096: first host-residual out position
RES_IN0 = RES_LO - (WMAX - 1) * C - C + 1  # input support start (3965)

NOUT = NT_DEV // NCORES   # 4 out-tiles per core
RC = R                    # rows per core (no row split)
NCHUNK = RC // CHUNK

# fp8 halo: true band support beyond a 512-lin group is <= 96 below and
# <= 92 above (asserted on the host against A).
HLO = 96
HHI = 92

# True partition width of each d=2 weight block (max over the 8 lin-groups,
# from the mel band layout; asserted on the host against A).  d=2 blocks
# read the FIRST W partitions of the tile above, so they run at base
# partition 0 with a partial span.  d=0 blocks read the LAST W partitions
# of the tile below; non-zero base partitions only allow short spans on the
# PE, so those stay full-width.
D2W = {0: 68, 1: 104, 2: 88}

_prog_cache = {}


def _build_program(loop_iters=1, unroll=1):
    """Uniform SPMD program: per core, NOUT out-tiles x 3 diagonals of
    [128,128] matmuls over [*,512] row chunks; halo diagonals in fp8.

    loop_iters > 1 wraps the body in a For_i replay loop (timing vehicle).
    `unroll` emits that many bodies per For_i iteration so the loop's
    all-engine back-edge barrier amortizes; loop_iters % unroll == 0."""
    import concourse.bacc as bacc
    import concourse.tile as tile
    import concourse.mybir as mybir

    key = (loop_iters, unroll)
    if key in _prog_cache:
        return _prog_cache[key]

    f32 = mybir.dt.float32
    f16 = mybir.dt.float16
    f8 = mybir.dt.float8e4

    nc = bacc.Bacc("TRN2", target_bir_lowering=False, debug=False,
                   num_devices=NCORES)
    xin = nc.dram_tensor("xin", [NOUT * 128, RC], f16,
                         kind="ExternalInput").ap()
    x8lo_d = nc.dram_tensor("x8lo", [HLO, RC], f8,
                            kind="ExternalInput").ap()
    x8hi_d = nc.dram_tensor("x8hi", [HHI, RC], f8,
                            kind="ExternalInput").ap()
    wts = nc.dram_tensor("wts", [128, (2 * NOUT - 1) * 128], f16,
                         kind="ExternalInput").ap()
    wd2a = nc.dram_tensor("wd2a", [D2W[0], 128], f16,
                          kind="ExternalInput").ap()
    wd2b = nc.dram_tensor("wd2b", [D2W[1], 2 * 128], f16,
                          kind="ExternalInput").ap()
    w8 = nc.dram_tensor("w8", [HLO, 2 * 128], f8,
                        kind="ExternalInput").ap()
    out = nc.dram_tensor("out", [NOUT * 128, RC], f16,
                         kind="ExternalOutput").ap()

    with tile.TileContext(nc) as tc:
        with (
            tc.tile_pool(name="xp", bufs=2) as xp,
            tc.tile_pool(name="wp", bufs=2) as wp,
            tc.tile_pool(name="yp", bufs=4) as yp,
            tc.tile_pool(name="pp", bufs=2, space="PSUM") as pp,
        ):
            def dma_in():
                # DMA issue order feeds the compute stream: out-tile 0 only
                # needs weights+x0+x1(+halo), so it starts while x2/x3 stream.
                # wts carries the 4 main-diagonal blocks + the 3 full-width
                # d0 blocks (out-tiles 1..3)
                wt = wp.tile([128, (2 * NOUT - 1) * 128], f16, tag="w",
                             name="wt")
                nc.sync.dma_start(wt[:], wts)
                w2a = wp.tile([D2W[0], 128], f16, tag="w2a", name="w2a")
                nc.sync.dma_start(w2a[:], wd2a)
                w2b = wp.tile([D2W[1], 2 * 128], f16, tag="w2b", name="w2b")
                nc.sync.dma_start(w2b[:], wd2b)
                xs = []
                for i in range(NOUT):
                    t = xp.tile([128, RC], f16, tag=f"x{i}", name=f"x{i}")
                    xs.append(t)
                for i in (0, 1):
                    nc.sync.dma_start(xs[i][:], xin[i * 128:(i + 1) * 128, :])
                w8t = wp.tile([HLO, 2 * 128], f8, tag="w8", name="w8t")
                nc.sync.dma_start(w8t[:], w8)
                x8lo = xp.tile([HLO, RC], f8, tag="x8lo", name="x8lo")
                nc.sync.dma_start(x8lo[:], x8lo_d)
                x8hi = xp.tile([HHI, RC], f8, tag="x8hi", name="x8hi")
                nc.sync.dma_start(x8hi[:], x8hi_d)
                for i in range(2, NOUT):
                    nc.sync.dma_start(xs[i][:], xin[i * 128:(i + 1) * 128, :])
                return (wt, w2a, w2b, w8t, xs, x8lo, x8hi)

            def compute(handles):
                wt, w2a, w2b, w8t, xs, x8lo, x8hi = handles

                def operands(j, d, c0, c1):
                    if d == 1:
                        return (wt[:, j * 128:(j + 1) * 128],
                                xs[j][:, c0:c1])
                    if d == 0:
                        if j == 0:
                            return w8t[0:HLO, 0:128], x8lo[:, c0:c1]
                        return (wt[:, (NOUT + j - 1) * 128:
                                    (NOUT + j) * 128],
                                xs[j - 1][:, c0:c1])
                    if j == NOUT - 1:
                        return w8t[0:HHI, 128:256], x8hi[:, c0:c1]
                    w = D2W[j]
                    lhsT = (w2a[:, 0:128] if j == 0 else
                            w2b[:, (j - 1) * 128:j * 128])
                    return lhsT[0:w, :], xs[j + 1][0:w, c0:c1]

                # (j, d) blocks in data-arrival order: each block is 4
                # same-stationary matmuls; two out-tiles accumulate in
                # flight (8 PSUM banks), so the PE never waits for x DMA.
                sched = [(0, 1), (1, 0), (0, 2), (1, 1), (0, 0), (1, 2),
                         (2, 0), (2, 1), (3, 0), (2, 2), (3, 1), (3, 2)]
                first = {j: min(i for i, (jj, _) in enumerate(sched) if jj == j)
                         for j in range(NOUT)}
                last = {j: max(i for i, (jj, _) in enumerate(sched) if jj == j)
                        for j in range(NOUT)}
                pss = {}
                for si, (j, d) in enumerate(sched):
                    if si == first[j]:
                        pss[j] = [pp.tile([128, CHUNK], f32, tag=f"ps{ch}",
                                          name=f"ps{ch}")
                                  for ch in range(NCHUNK)]
                    for ch in range(NCHUNK):
                        lhsT, rhs = operands(j, d, ch * CHUNK,
                                             (ch + 1) * CHUNK)
                        nc.tensor.matmul(pss[j][ch][:], lhsT, rhs,
                                         start=(si == first[j]),
                                         stop=(si == last[j]))
                    if si == last[j]:
                        y = yp.tile([128, RC], f16, tag="y", name="y")
                        for ch in range(NCHUNK):
                            dst = y[:, ch * CHUNK:(ch + 1) * CHUNK]
                            if (j * NCHUNK + ch) % 2 == 0:
                                nc.scalar.copy(dst, pss[j][ch][:])
                            else:
                                nc.vector.tensor_copy(dst, pss[j][ch][:])
                        nc.sync.dma_start(out[j * 128:(j + 1) * 128, :], y[:])

            if loop_iters == 1:
                compute(dma_in())
            else:
                # U bodies per For_i iteration: amortizes the loop's
                # all-engine barrier; next body's input DMAs issue before
                # the current body's compute so transfers stay back-to-back.
                assert loop_iters % unroll == 0
                with tc.For_i(0, loop_iters // unroll, 1) as _i:
                    h = dma_in()
                    for _u in range(unroll):
                        nh = dma_in() if _u < unroll - 1 else None
                        compute(h)
                        h = nh

    nc.compile()
    _prog_cache[key] = nc
    return nc


def _build_A(pre_weight, pre_bias, post_weight, post_bias, mask, ola_window,
             f_idxes):
    """Host: banded operator A[in_lin, out_lin] (LPAD x LPAD, fp32) and the
    constant bias image (C, F)."""
    fi = f_idxes.reshape(KB, WMAX).astype(np.int64)
    mk = mask.reshape(KB, WMAX).astype(np.float32)
    ola = ola_window.astype(np.float32)

    # effective per-band operators with mask and 1/ola folded in
    # row (input) index i = w*C + c ; col (output) index j = w'*C + c'
    mrow = np.repeat(mk, C, axis=1)                     # (KB, WMAX*C)
    inv_ola = np.where(ola != 0, 1.0 / ola, 0.0)
    ola_cols = inv_ola[fi]                              # (KB, WMAX)
    mcol = np.repeat(mk * ola_cols, C, axis=1)          # (KB, WMAX*C)

    w1 = pre_weight * mrow[:, :, None]                  # (KB, D, 128)
    w2 = post_weight * mcol[:, None, :]                 # (KB, 128, D)
    Mk = np.matmul(w1, w2)                              # (KB, D, D) fp32

    A = np.zeros((LPAD, LPAD), np.float32)
    lin = (fi[:, :, None] * C + np.arange(C)[None, None, :]).reshape(KB, -1)
    for k in range(KB):
        idx = lin[k]
        A[np.ix_(idx, idx)] += Mk[k]   # duplicate idx entries are all-zero rows/cols

    # bias: (pre_bias @ W2_raw + post_bias) * mask / ola, scattered -> (C, F)
    by = (np.einsum('ko,koj->kj', pre_bias, post_weight) + post_bias)  # (KB, D)
    by = by * mcol                                                      # masked + /ola
    bias_img = np.zeros((C, F), np.float32)
    np.add.at(bias_img,
              (np.tile(np.arange(C), (KB, WMAX, 1)).reshape(KB, -1),
               np.repeat(fi, C, axis=1)),
              by)
    return A, bias_img


def _to_f8(a):
    """TRN FP8_EXP4 (E4M3 with +-240 max) == ml_dtypes.float8_e4m3."""
    import ml_dtypes
    return np.clip(a, -240.0, 240.0).astype(ml_dtypes.float8_e4m3)


def _shard_inputs(x, A):
    """Per-core {xin, x8in, wts, w8} arrays + the host residual rows."""
    # x (B, C, T, F) -> X_lin [L, R], lin = f*4+c, r = b*T+t
    X = np.ascontiguousarray(
        x.transpose(3, 1, 0, 2).reshape(L, R).astype(np.float32))
    # halo-padded copies: HLO zero rows on top, zeros to lin 4096+HHI below
    Xp = np.zeros((HLO + RES_LO + HHI, R), np.float32)
    Xp[HLO:HLO + L] = X
    Ap = np.zeros((HLO + RES_LO + HHI, LPAD), np.float32)
    Ap[HLO:HLO + L] = A[:L]

    in_maps = []
    for cid in range(NCORES):
        g0 = cid * NOUT * 128            # first owned out lin
        # owned fp16 tiles
        xin_a = X[g0:g0 + NOUT * 128].astype(np.float16)
        # fp8 halo rows [g0-HLO, g0) and [g0+512, g0+512+HHI)
        x8lo_a = Xp[g0:g0 + HLO]
        x8hi_a = Xp[HLO + g0 + NOUT * 128:HLO + g0 + NOUT * 128 + HHI]
        # partial-width off-diagonal blocks: d=0 reads the last W rows of
        # the tile below, d=2 the first W rows of the tile above
        def d0_block(j, w):
            o0 = g0 + j * 128
            blk = Ap[HLO + o0 - w:HLO + o0, o0:o0 + 128]
            assert np.all(Ap[HLO + o0 - 128:HLO + o0 - w, o0:o0 + 128] == 0.0)
            return blk

        def d2_block(j, w):
            o0 = g0 + j * 128
            i0 = o0 + 128
            blk = Ap[HLO + i0:HLO + i0 + w, o0:o0 + 128]
            assert np.all(Ap[HLO + i0 + w:HLO + i0 + 128, o0:o0 + 128] == 0.0)
            return blk

        def pad_rows(blk, rows):
            out = np.zeros((rows, blk.shape[1]), np.float32)
            out[:blk.shape[0]] = blk
            return out

        # fp16 main-diagonal blocks + the 3 full-width d0 blocks
        wts_a = np.zeros((128, (2 * NOUT - 1) * 128), np.float32)
        for j in range(NOUT):
            o0 = g0 + j * 128
            wts_a[:, j * 128:(j + 1) * 128] = Ap[HLO + o0:HLO + o0 + 128,
                                                 o0:o0 + 128]
        for j in range(1, NOUT):
            wts_a[:, (NOUT + j - 1) * 128:(NOUT + j) * 128] = d0_block(j, 128)
        wd2a_a = d2_block(0, D2W[0])
        wd2b_a = np.concatenate([pad_rows(d2_block(1, D2W[1]), D2W[1]),
                                 pad_rows(d2_block(2, D2W[2]), D2W[1])],
                                axis=1)
        # fp8 halo weight blocks
        w8_a = np.zeros((HLO, 2 * 128), np.float32)
        w8_a[:, :128] = Ap[g0:g0 + HLO, g0:g0 + 128]
        w8_a[:HHI, 128:] = Ap[HLO + g0 + NOUT * 128:
                              HLO + g0 + NOUT * 128 + HHI,
                              g0 + (NOUT - 1) * 128:g0 + NOUT * 128]
        # halo support must fit (A is banded; verified against the real A)
        assert np.all(Ap[:g0, g0:g0 + NOUT * 128] == 0.0)
        assert np.all(Ap[HLO + g0 + NOUT * 128 + HHI:,
                         g0:g0 + NOUT * 128] == 0.0)
        in_maps.append({
            "xin": np.ascontiguousarray(xin_a),
            "x8lo": np.ascontiguousarray(_to_f8(x8lo_a)),
            "x8hi": np.ascontiguousarray(_to_f8(x8hi_a)),
            "wts": np.ascontiguousarray(wts_a.astype(np.float16)),
            "wd2a": np.ascontiguousarray(wd2a_a.astype(np.float16)),
            "wd2b": np.ascontiguousarray(wd2b_a.astype(np.float16)),
            "w8": np.ascontiguousarray(_to_f8(w8_a)),
        })

    # host residual: the 4 real out positions of lin-tile 32 (f-bin 1024)
    residual = A[RES_IN0:L, RES_LO:L].T @ X[RES_IN0:L]    # [4, R] fp32
    return in_maps, residual


def _gather_output(results, bias_img, residual):
    out_lin = np.zeros((LPAD, R), np.float32)
    for cid in range(NCORES):
        g0 = cid * NOUT * 128
        out_lin[g0:g0 + NOUT * 128] = results[cid]["out"].astype(np.float32)
    out_lin[RES_LO:L] = residual
    # [L, R] -> (B, C, T, F):  lin = f*4+c, r = b*T+t
    out = out_lin[:L].reshape(F, C, B, T).transpose(2, 1, 3, 0)
    out = np.ascontiguousarray(out) + bias_img[None, :, None, :]
    return out.astype(np.float32)


def _run_on_device(in_maps, loop_iters=1, unroll=1):
    from concourse.bass_utils import run_bass_kernel_spmd
    nc = _build_program(loop_iters, unroll)
    res = run_bass_kernel_spmd(nc, in_maps, list(range(NCORES)))
    return res.results


def kernel(x, pre_weight, pre_bias, post_weight, post_bias, mask, ola_window,
           f_idxes):
    x = np.asarray(x, np.float32)
    pre_weight = np.asarray(pre_weight, np.float32)
    pre_bias = np.asarray(pre_bias, np.float32)
    post_weight = np.asarray(post_weight, np.float32)
    post_bias = np.asarray(post_bias, np.float32)
    mask = np.asarray(mask, np.float32)
    ola_window = np.asarray(ola_window, np.float32)
    f_idxes = np.asarray(f_idxes)

    A, bias_img = _build_A(pre_weight, pre_bias, post_weight, post_bias,
                           mask, ola_window, f_idxes)
    in_maps, residual = _shard_inputs(x, A)
    results = _run_on_device(in_maps)
    return _gather_output(results, bias_img, residual)


# revision 41
# speedup vs baseline: 1.0362x; 1.0362x over previous
"""Trainium2 Bass kernel for nn_BandSplit (grouped band einsum as banded matmul).

The reference computes, per (b, t) row:
    g = gather(x, f_idxes) * mask            # per-band slice of the spectrum
    h = einsum('ki,kio->ko', g, pre_weight) + pre_bias
    y = einsum('ko,koj->kj', h, post_weight) + post_bias
    out = scatter_add(y * mask) / ola_window

Because each band's nonzero bins are a contiguous f-range, the whole pipeline
is linear in x and collapses to ONE banded matrix multiply in the interleaved
linear space  lin = f*4 + c  (bandwidth <= 131 < 132):

    out_lin[l', r] = sum_l A[l, l'] * x_lin[l, r]
    A = sum_k scatter(diag(mask_k) @ W1_k @ W2_k @ diag(mask_k / ola))

A is built on the host from the (small) weight inputs.  x is pre-transposed on
the host into [lin, rows] tiles so the device does only contiguous DMA plus
dense 128x128x512 matmuls (fp32 PSUM accumulation) on 3 block-diagonals.

Sharding: 8 lin-groups of 4 out-tiles (of 128) x full rows, one per core.
Each out-tile j contracts its own tile (d=1) and both neighbours (d=0/d=2).
The group-edge halo paths carry only ~5% of the output energy, so they run
in fp8 (E4M3) at their true support width (<=112 below / <=104 above the
group), saving ~45% of the halo DMA bytes; everything else is fp16 with
fp32 PSUM.  Measured end-to-end error ~8e-3 vs the fp32 reference.

DMA layout is chosen for few, large transfers (the SP sequencer pays ~650ns
per issued DMA): one DMA per x tile / weight blob / out tile.  Matmuls are
ordered (out-tile, diagonal, chunk) so the PE keeps one stationary weight
block for 4 consecutive matmuls.
"""

import numpy as np

# ---- problem constants (hardcoded; harness supplies matching inputs) ----
B, C, T, F = 4, 4, 512, 1025
KB, WMAX = 256, 33
L = F * C                 # 4100 linear positions
NT = (L + 127) // 128     # 33 tiles of 128
LPAD = NT * 128           # 4224
R = B * T                 # 2048 rows (b, t)
NCORES = 8
ND = 3                    # block diagonals
CHUNK = 512               # PSUM bank (fp32) free-dim limit

# The last lin-tile (32) covers only 4 real positions (f-bin 1024); its
# output is computed on the host, so the device grid is exactly 32 tiles.
NT_DEV = 32
RES_LO = NT_DEV * 128            # 4096: first host-residual out position
RES_IN0 = RES_LO - (WMAX - 1) * C - C + 1  # input support start (3965)

NOUT = NT_DEV // NCORES   # 4 out-tiles per core
RC = R                    # rows per core (no row split)
NCHUNK = RC // CHUNK

# fp8 halo: true band support beyond a 512-lin group is <= 96 below and
# <= 92 above (asserted on the host against A).
HLO = 96
HHI = 92



_prog_cache = {}


def _build_program(loop_iters=1, unroll=1):
    """Uniform SPMD program: per core, NOUT out-tiles x 3 diagonals of
    [128,128] matmuls over [*,512] row chunks; halo diagonals in fp8.

    loop_iters > 1 wraps the body in a For_i replay loop (timing vehicle).
    `unroll` emits that many bodies per For_i iteration so the loop's
    all-engine back-edge barrier amortizes; loop_iters % unroll == 0."""
    import concourse.bacc as bacc
    import concourse.tile as tile
    import concourse.mybir as mybir

    key = (loop_iters, unroll)
    if key in _prog_cache:
        return _prog_cache[key]

    f32 = mybir.dt.float32
    f16 = mybir.dt.float16
    f8 = mybir.dt.float8e4

    nc = bacc.Bacc("TRN2", target_bir_lowering=False, debug=False,
                   num_devices=NCORES)
    # per-core DMA count dominates the steady-state on HW (~0.8us fixed
    # cost per issued DMA), so everything is packed into 3 transfers: one
    # fp16 blob (owned x tiles side-by-side in the free dim + all fp16
    # weight blocks), one fp8 blob (both halos + halo weights), one output.
    xin = nc.dram_tensor("xin", [128, NOUT * RC + 10 * 128], f16,
                         kind="ExternalInput").ap()
    x8in = nc.dram_tensor("x8in", [HLO, 2 * RC + 2 * 128], f8,
                          kind="ExternalInput").ap()
    out = nc.dram_tensor("out", [128, NOUT * RC], f16,
                         kind="ExternalOutput").ap()

    with tile.TileContext(nc) as tc:
        with (
            tc.tile_pool(name="xp", bufs=2) as xp,
            tc.tile_pool(name="wp", bufs=2) as wp,
            tc.tile_pool(name="yp", bufs=2) as yp,
            tc.tile_pool(name="pp", bufs=2, space="PSUM") as pp,
        ):
            def dma_in():
                xt = xp.tile([128, NOUT * RC + 10 * 128], f16, tag="x",
                             name="xt")
                nc.sync.dma_start(xt[:], xin)
                x8t = xp.tile([HLO, 2 * RC + 2 * 128], f8, tag="x8",
                              name="x8t")
                nc.sync.dma_start(x8t[:], x8in)
                return (xt, x8t)

            def compute(handles):
                xt, x8t = handles
                W0 = NOUT * RC           # fp16 weight blob column offset
                W8 = 2 * RC              # fp8 halo-weight column offset

                # fp16 weight blob columns: d1 j0..3 | d0 j1..3 | d2 j0..2
                def operands(j, d, c0, c1):
                    if d == 1:
                        return (xt[:, W0 + j * 128:W0 + (j + 1) * 128],
                                xt[:, j * RC + c0:j * RC + c1])
                    if d == 0:
                        if j == 0:
                            return (x8t[0:HLO, W8:W8 + 128],
                                    x8t[0:HLO, c0:c1])
                        return (xt[:, W0 + (3 + j) * 128:
                                   W0 + (4 + j) * 128],
                                xt[:, (j - 1) * RC + c0:(j - 1) * RC + c1])
                    if j == NOUT - 1:
                        return (x8t[0:HHI, W8 + 128:W8 + 256],
                                x8t[0:HHI, RC + c0:RC + c1])
                    return (xt[:, W0 + (7 + j) * 128:W0 + (8 + j) * 128],
                            xt[:, (j + 1) * RC + c0:(j + 1) * RC + c1])

                # (j, d) blocks in data-arrival order: each block is 4
                # same-stationary matmuls; two out-tiles accumulate in
                # flight (8 PSUM banks), so the PE never waits for x DMA.
                sched = [(0, 1), (1, 0), (0, 2), (1, 1), (0, 0), (1, 2),
                         (2, 0), (2, 1), (3, 0), (2, 2), (3, 1), (3, 2)]
                first = {j: min(i for i, (jj, _) in enumerate(sched) if jj == j)
                         for j in range(NOUT)}
                last = {j: max(i for i, (jj, _) in enumerate(sched) if jj == j)
                        for j in range(NOUT)}
                pss = {}
                handles_y = {}
                for si, (j, d) in enumerate(sched):
                    if si == first[j]:
                        pss[j] = [pp.tile([128, CHUNK], f32, tag=f"ps{ch}",
                                          name=f"ps{ch}")
                                  for ch in range(NCHUNK)]
                    for ch in range(NCHUNK):
                        lhsT, rhs = operands(j, d, ch * CHUNK,
                                             (ch + 1) * CHUNK)
                        nc.tensor.matmul(pss[j][ch][:], lhsT, rhs,
                                         start=(si == first[j]),
                                         stop=(si == last[j]))
                    if si == last[j]:
                        if j == 0:
                            handles_y[0] = yp.tile([128, NOUT * RC], f16,
                                                   tag="y", name="y")
                        y = handles_y[0]
                        for ch in range(NCHUNK):
                            o0 = j * RC + ch * CHUNK
                            dst = y[:, o0:o0 + CHUNK]
                            if (j * NCHUNK + ch) % 2 == 0:
                                nc.scalar.copy(dst, pss[j][ch][:])
                            else:
                                nc.vector.tensor_copy(dst, pss[j][ch][:])
                        if j == NOUT - 1:
                            nc.sync.dma_start(out, y[:])

            if loop_iters == 1:
                compute(dma_in())
            else:
                # U bodies per For_i iteration: amortizes the loop's
                # all-engine barrier; next body's input DMAs issue before
                # the current body's compute so transfers stay back-to-back.
                assert loop_iters % unroll == 0
                with tc.For_i(0, loop_iters // unroll, 1) as _i:
                    h = dma_in()
                    for _u in range(unroll):
                        nh = dma_in() if _u < unroll - 1 else None
                        compute(h)
                        h = nh

    nc.compile()
    _prog_cache[key] = nc
    return nc


def _build_A(pre_weight, pre_bias, post_weight, post_bias, mask, ola_window,
             f_idxes):
    """Host: banded operator A[in_lin, out_lin] (LPAD x LPAD, fp32) and the
    constant bias image (C, F)."""
    fi = f_idxes.reshape(KB, WMAX).astype(np.int64)
    mk = mask.reshape(KB, WMAX).astype(np.float32)
    ola = ola_window.astype(np.float32)

    # effective per-band operators with mask and 1/ola folded in
    # row (input) index i = w*C + c ; col (output) index j = w'*C + c'
    mrow = np.repeat(mk, C, axis=1)                     # (KB, WMAX*C)
    inv_ola = np.where(ola != 0, 1.0 / ola, 0.0)
    ola_cols = inv_ola[fi]                              # (KB, WMAX)
    mcol = np.repeat(mk * ola_cols, C, axis=1)          # (KB, WMAX*C)

    w1 = pre_weight * mrow[:, :, None]                  # (KB, D, 128)
    w2 = post_weight * mcol[:, None, :]                 # (KB, 128, D)
    Mk = np.matmul(w1, w2)                              # (KB, D, D) fp32

    A = np.zeros((LPAD, LPAD), np.float32)
    lin = (fi[:, :, None] * C + np.arange(C)[None, None, :]).reshape(KB, -1)
    for k in range(KB):
        idx = lin[k]
        A[np.ix_(idx, idx)] += Mk[k]   # duplicate idx entries are all-zero rows/cols

    # bias: (pre_bias @ W2_raw + post_bias) * mask / ola, scattered -> (C, F)
    by = (np.einsum('ko,koj->kj', pre_bias, post_weight) + post_bias)  # (KB, D)
    by = by * mcol                                                      # masked + /ola
    bias_img = np.zeros((C, F), np.float32)
    np.add.at(bias_img,
              (np.tile(np.arange(C), (KB, WMAX, 1)).reshape(KB, -1),
               np.repeat(fi, C, axis=1)),
              by)
    return A, bias_img


def _to_f8(a):
    """TRN FP8_EXP4 (E4M3 with +-240 max) == ml_dtypes.float8_e4m3."""
    import ml_dtypes
    return np.clip(a, -240.0, 240.0).astype(ml_dtypes.float8_e4m3)


def _shard_inputs(x, A):
    """Per-core {xin, x8in} blobs + the host residual rows."""
    # x (B, C, T, F) -> X_lin [L, R], lin = f*4+c, r = b*T+t
    X = np.ascontiguousarray(
        x.transpose(3, 1, 0, 2).reshape(L, R).astype(np.float32))
    # halo-padded copies: HLO zero rows on top, zeros to lin 4096+HHI below
    Xp = np.zeros((HLO + RES_LO + HHI, R), np.float32)
    Xp[HLO:HLO + L] = X
    Ap = np.zeros((HLO + RES_LO + HHI, LPAD), np.float32)
    Ap[HLO:HLO + L] = A[:L]

    in_maps = []
    for cid in range(NCORES):
        g0 = cid * NOUT * 128            # first owned out lin
        # owned fp16 tiles, packed side-by-side along the free dim
        xin_a = np.concatenate(
            [X[g0 + i * 128:g0 + (i + 1) * 128] for i in range(NOUT)],
            axis=1).astype(np.float16)
        # fp8 halo rows [g0-HLO, g0) and [g0+512, g0+512+HHI), packed
        x8_a = np.zeros((HLO, 2 * R), np.float32)
        x8_a[:, :R] = Xp[g0:g0 + HLO]
        x8_a[:HHI, R:] = Xp[HLO + g0 + NOUT * 128:
                            HLO + g0 + NOUT * 128 + HHI]
        # fp16 weight blob: d1 j0..3 | d0 j1..3 | d2 j0..2 (full width)
        wts_a = np.zeros((128, 10 * 128), np.float32)
        for j in range(NOUT):
            o0 = g0 + j * 128
            wts_a[:, j * 128:(j + 1) * 128] = Ap[HLO + o0:HLO + o0 + 128,
                                                 o0:o0 + 128]
        for j in range(1, NOUT):
            o0 = g0 + j * 128
            wts_a[:, (3 + j) * 128:(4 + j) * 128] = \
                Ap[HLO + o0 - 128:HLO + o0, o0:o0 + 128]
        for j in range(NOUT - 1):
            o0 = g0 + j * 128
            wts_a[:, (7 + j) * 128:(8 + j) * 128] = \
                Ap[HLO + o0 + 128:HLO + o0 + 256, o0:o0 + 128]
        # fp8 halo weight blocks
        w8_a = np.zeros((HLO, 2 * 128), np.float32)
        w8_a[:, :128] = Ap[g0:g0 + HLO, g0:g0 + 128]
        w8_a[:HHI, 128:] = Ap[HLO + g0 + NOUT * 128:
                              HLO + g0 + NOUT * 128 + HHI,
                              g0 + (NOUT - 1) * 128:g0 + NOUT * 128]
        # halo support must fit (A is banded; verified against the real A)
        assert np.all(Ap[:g0, g0:g0 + NOUT * 128] == 0.0)
        assert np.all(Ap[HLO + g0 + NOUT * 128 + HHI:,
                         g0:g0 + NOUT * 128] == 0.0)
        in_maps.append({
            "xin": np.ascontiguousarray(
                np.concatenate([xin_a, wts_a.astype(np.float16)], axis=1)),
            "x8in": np.ascontiguousarray(
                np.concatenate([_to_f8(x8_a), _to_f8(w8_a)], axis=1)),
        })

    # host residual: the 4 real out positions of lin-tile 32 (f-bin 1024)
    residual = A[RES_IN0:L, RES_LO:L].T @ X[RES_IN0:L]    # [4, R] fp32
    return in_maps, residual


def _gather_output(results, bias_img, residual):
    out_lin = np.zeros((LPAD, R), np.float32)
    for cid in range(NCORES):
        g0 = cid * NOUT * 128
        o = results[cid]["out"].astype(np.float32)   # [128, NOUT*R]
        for j in range(NOUT):
            out_lin[g0 + j * 128:g0 + (j + 1) * 128] = \
                o[:, j * R:(j + 1) * R]
    out_lin[RES_LO:L] = residual
    # [L, R] -> (B, C, T, F):  lin = f*4+c, r = b*T+t
    out = out_lin[:L].reshape(F, C, B, T).transpose(2, 1, 3, 0)
    out = np.ascontiguousarray(out) + bias_img[None, :, None, :]
    return out.astype(np.float32)


def _run_on_device(in_maps, loop_iters=1, unroll=1):
    from concourse.bass_utils import run_bass_kernel_spmd
    nc = _build_program(loop_iters, unroll)
    res = run_bass_kernel_spmd(nc, in_maps, list(range(NCORES)))
    return res.results


def kernel(x, pre_weight, pre_bias, post_weight, post_bias, mask, ola_window,
           f_idxes):
    x = np.asarray(x, np.float32)
    pre_weight = np.asarray(pre_weight, np.float32)
    pre_bias = np.asarray(pre_bias, np.float32)
    post_weight = np.asarray(post_weight, np.float32)
    post_bias = np.asarray(post_bias, np.float32)
    mask = np.asarray(mask, np.float32)
    ola_window = np.asarray(ola_window, np.float32)
    f_idxes = np.asarray(f_idxes)

    A, bias_img = _build_A(pre_weight, pre_bias, post_weight, post_bias,
                           mask, ola_window, f_idxes)
    in_maps, residual = _shard_inputs(x, A)
    results = _run_on_device(in_maps)
    return _gather_output(results, bias_img, residual)


# revision 42
# speedup vs baseline: 1.1100x; 1.0712x over previous
"""Trainium2 Bass kernel for nn_BandSplit (grouped band einsum as banded matmul).

The reference computes, per (b, t) row:
    g = gather(x, f_idxes) * mask            # per-band slice of the spectrum
    h = einsum('ki,kio->ko', g, pre_weight) + pre_bias
    y = einsum('ko,koj->kj', h, post_weight) + post_bias
    out = scatter_add(y * mask) / ola_window

Because each band's nonzero bins are a contiguous f-range, the whole pipeline
is linear in x and collapses to ONE banded matrix multiply in the interleaved
linear space  lin = f*4 + c  (bandwidth <= 131 < 132):

    out_lin[l', r] = sum_l A[l, l'] * x_lin[l, r]
    A = sum_k scatter(diag(mask_k) @ W1_k @ W2_k @ diag(mask_k / ola))

A is built on the host from the (small) weight inputs.  x is pre-transposed on
the host into [lin, rows] tiles so the device does only contiguous DMA plus
dense 128x128x512 matmuls (fp32 PSUM accumulation) on 3 block-diagonals.

Sharding: 8 lin-groups of 4 out-tiles (of 128) x full rows, one per core.
Each out-tile j contracts its own tile (d=1) and both neighbours (d=0/d=2).
The group-edge halo paths carry only ~5% of the output energy, so they run
in fp8 (E4M3) at their true support width (<=112 below / <=104 above the
group), saving ~45% of the halo DMA bytes; everything else is fp16 with
fp32 PSUM.  Measured end-to-end error ~8e-3 vs the fp32 reference.

DMA layout is chosen for few, large transfers (the SP sequencer pays ~650ns
per issued DMA): one DMA per x tile / weight blob / out tile.  Matmuls are
ordered (out-tile, diagonal, chunk) so the PE keeps one stationary weight
block for 4 consecutive matmuls.
"""

import numpy as np

# ---- problem constants (hardcoded; harness supplies matching inputs) ----
B, C, T, F = 4, 4, 512, 1025
KB, WMAX = 256, 33
L = F * C                 # 4100 linear positions
NT = (L + 127) // 128     # 33 tiles of 128
LPAD = NT * 128           # 4224
R = B * T                 # 2048 rows (b, t)
NCORES = 8
ND = 3                    # block diagonals
CHUNK = 512               # PSUM bank (fp32) free-dim limit

# The last lin-tile (32) covers only 4 real positions (f-bin 1024); its
# output is computed on the host, so the device grid is exactly 32 tiles.
NT_DEV = 32
RES_LO = NT_DEV * 128            # 4096: first host-residual out position
RES_IN0 = RES_LO - (WMAX - 1) * C - C + 1  # input support start (3965)

NOUT = NT_DEV // NCORES   # 4 out-tiles per core
RC = R                    # rows per core (no row split)
NCHUNK = RC // CHUNK

# fp8 halo: true band support beyond a 512-lin group is <= 112 below and
# <= 104 above (asserted on the host against A).  Both halves are packed in
# one [HLO, 2*RC] fp8 tile: cols [0,RC) = lower halo, cols [RC,2RC) = upper.
HLO = 112
HHI = 104

# fp16 weight blocks, in (out-tile j, diagonal d) issue order; the two halo
# blocks (0,0) and (NOUT-1,2) live in the fp8 blob instead.
WBLOCKS = [(j, d) for j in range(NOUT) for d in range(ND)
           if (j, d) not in ((0, 0), (NOUT - 1, 2))]

_prog_cache = {}


def _build_program(loop_iters=1, unroll=1):
    """Uniform SPMD program: per core, NOUT out-tiles x 3 diagonals of
    [128,128] matmuls over [*,512] row chunks; halo diagonals in fp8.

    loop_iters > 1 wraps the body in a For_i replay loop (timing vehicle).
    `unroll` emits that many bodies per For_i iteration so the loop's
    all-engine back-edge barrier amortizes; loop_iters % unroll == 0."""
    import concourse.bacc as bacc
    import concourse.tile as tile
    import concourse.mybir as mybir

    key = (loop_iters, unroll)
    if key in _prog_cache:
        return _prog_cache[key]

    f32 = mybir.dt.float32
    f16 = mybir.dt.float16
    f8 = mybir.dt.float8e4

    nc = bacc.Bacc("TRN2", target_bir_lowering=False, debug=False,
                   num_devices=NCORES)
    xin = nc.dram_tensor("xin", [NOUT * 128, RC], f16,
                         kind="ExternalInput").ap()
    x8in = nc.dram_tensor("x8in", [HLO, 2 * RC], f8,
                          kind="ExternalInput").ap()
    wts = nc.dram_tensor("wts", [128, len(WBLOCKS) * 128], f16,
                         kind="ExternalInput").ap()
    w8 = nc.dram_tensor("w8", [HLO, 2 * 128], f8,
                        kind="ExternalInput").ap()
    out = nc.dram_tensor("out", [NOUT * 128, RC], f16,
                         kind="ExternalOutput").ap()

    with tile.TileContext(nc) as tc:
        with (
            tc.tile_pool(name="xp", bufs=2) as xp,
            tc.tile_pool(name="wp", bufs=2) as wp,
            tc.tile_pool(name="yp", bufs=4) as yp,
            tc.tile_pool(name="pp", bufs=2, space="PSUM") as pp,
        ):
            def dma_in():
                # DMA issue order feeds the compute stream: out-tile 0 only
                # needs wt+x0+x1(+halo), so it can start while x2/x3 stream.
                wt = wp.tile([128, len(WBLOCKS) * 128], f16, tag="w",
                             name="wt")
                nc.sync.dma_start(wt[:], wts)
                xs = []
                for i in range(NOUT):
                    t = xp.tile([128, RC], f16, tag=f"x{i}", name=f"x{i}")
                    xs.append(t)
                for i in (0, 1):
                    nc.sync.dma_start(xs[i][:], xin[i * 128:(i + 1) * 128, :])
                w8t = wp.tile([HLO, 2 * 128], f8, tag="w8", name="w8t")
                nc.sync.dma_start(w8t[:], w8)
                x8t = xp.tile([HLO, 2 * RC], f8, tag="x8", name="x8t")
                nc.sync.dma_start(x8t[:], x8in)
                for i in range(2, NOUT):
                    nc.sync.dma_start(xs[i][:], xin[i * 128:(i + 1) * 128, :])
                return wt, w8t, xs, x8t

            def compute(handles):
                wt, w8t, xs, x8t = handles
                # (j, d) blocks in data-arrival order: each block is 4
                # same-stationary matmuls; two out-tiles accumulate in
                # flight (8 PSUM banks), so the PE never waits for x DMA.
                sched = [(0, 1), (1, 0), (0, 2), (1, 1), (0, 0), (1, 2),
                         (2, 0), (2, 1), (3, 0), (2, 2), (3, 1), (3, 2)]
                first = {j: min(i for i, (jj, _) in enumerate(sched) if jj == j)
                         for j in range(NOUT)}
                last = {j: max(i for i, (jj, _) in enumerate(sched) if jj == j)
                        for j in range(NOUT)}
                pss = {}
                for si, (j, d) in enumerate(sched):
                    if si == first[j]:
                        pss[j] = [pp.tile([128, CHUNK], f32, tag=f"ps{ch}",
                                          name=f"ps{ch}")
                                  for ch in range(NCHUNK)]
                    if (j, d) == (0, 0):
                        lhsT = w8t[0:HLO, 0:128]
                    elif (j, d) == (NOUT - 1, 2):
                        lhsT = w8t[0:HHI, 128:256]
                    else:
                        bi = WBLOCKS.index((j, d))
                        lhsT = wt[:, bi * 128:(bi + 1) * 128]
                    for ch in range(NCHUNK):
                        c0 = ch * CHUNK
                        if (j, d) == (0, 0):
                            rhs = x8t[0:HLO, c0:c0 + CHUNK]
                        elif (j, d) == (NOUT - 1, 2):
                            rhs = x8t[0:HHI, RC + c0:RC + c0 + CHUNK]
                        else:
                            rhs = xs[j + d - 1][:, c0:c0 + CHUNK]
                        nc.tensor.matmul(pss[j][ch][:], lhsT, rhs,
                                         start=(si == first[j]),
                                         stop=(si == last[j]))
                    if si == last[j]:
                        y = yp.tile([128, RC], f16, tag="y", name="y")
                        for ch in range(NCHUNK):
                            dst = y[:, ch * CHUNK:(ch + 1) * CHUNK]
                            if (j * NCHUNK + ch) % 2 == 0:
                                nc.scalar.copy(dst, pss[j][ch][:])
                            else:
                                nc.vector.tensor_copy(dst, pss[j][ch][:])
                        nc.sync.dma_start(out[j * 128:(j + 1) * 128, :], y[:])

            if loop_iters == 1:
                compute(dma_in())
            else:
                # U bodies per For_i iteration: amortizes the loop's
                # all-engine barrier; next body's input DMAs issue before
                # the current body's compute so transfers stay back-to-back.
                assert loop_iters % unroll == 0
                with tc.For_i(0, loop_iters // unroll, 1) as _i:
                    h = dma_in()
                    for _u in range(unroll):
                        nh = dma_in() if _u < unroll - 1 else None
                        compute(h)
                        h = nh

    nc.compile()
    _prog_cache[key] = nc
    return nc


def _build_A(pre_weight, pre_bias, post_weight, post_bias, mask, ola_window,
             f_idxes):
    """Host: banded operator A[in_lin, out_lin] (LPAD x LPAD, fp32) and the
    constant bias image (C, F)."""
    fi = f_idxes.reshape(KB, WMAX).astype(np.int64)
    mk = mask.reshape(KB, WMAX).astype(np.float32)
    ola = ola_window.astype(np.float32)

    # effective per-band operators with mask and 1/ola folded in
    # row (input) index i = w*C + c ; col (output) index j = w'*C + c'
    mrow = np.repeat(mk, C, axis=1)                     # (KB, WMAX*C)
    inv_ola = np.where(ola != 0, 1.0 / ola, 0.0)
    ola_cols = inv_ola[fi]                              # (KB, WMAX)
    mcol = np.repeat(mk * ola_cols, C, axis=1)          # (KB, WMAX*C)

    w1 = pre_weight * mrow[:, :, None]                  # (KB, D, 128)
    w2 = post_weight * mcol[:, None, :]                 # (KB, 128, D)
    Mk = np.matmul(w1, w2)                              # (KB, D, D) fp32

    A = np.zeros((LPAD, LPAD), np.float32)
    lin = (fi[:, :, None] * C + np.arange(C)[None, None, :]).reshape(KB, -1)
    for k in range(KB):
        idx = lin[k]
        A[np.ix_(idx, idx)] += Mk[k]   # duplicate idx entries are all-zero rows/cols

    # bias: (pre_bias @ W2_raw + post_bias) * mask / ola, scattered -> (C, F)
    by = (np.einsum('ko,koj->kj', pre_bias, post_weight) + post_bias)  # (KB, D)
    by = by * mcol                                                      # masked + /ola
    bias_img = np.zeros((C, F), np.float32)
    np.add.at(bias_img,
              (np.tile(np.arange(C), (KB, WMAX, 1)).reshape(KB, -1),
               np.repeat(fi, C, axis=1)),
              by)
    return A, bias_img


def _to_f8(a):
    """TRN FP8_EXP4 (E4M3 with +-240 max) == ml_dtypes.float8_e4m3."""
    import ml_dtypes
    return np.clip(a, -240.0, 240.0).astype(ml_dtypes.float8_e4m3)


def _shard_inputs(x, A):
    """Per-core {xin, x8in, wts, w8} arrays + the host residual rows."""
    # x (B, C, T, F) -> X_lin [L, R], lin = f*4+c, r = b*T+t
    X = np.ascontiguousarray(
        x.transpose(3, 1, 0, 2).reshape(L, R).astype(np.float32))
    # halo-padded copies: HLO zero rows on top, zeros to lin 4096+HHI below
    Xp = np.zeros((HLO + RES_LO + HHI, R), np.float32)
    Xp[HLO:HLO + L] = X
    Ap = np.zeros((HLO + RES_LO + HHI, LPAD), np.float32)
    Ap[HLO:HLO + L] = A[:L]

    in_maps = []
    for cid in range(NCORES):
        g0 = cid * NOUT * 128            # first owned out lin
        # owned fp16 tiles
        xin_a = X[g0:g0 + NOUT * 128].astype(np.float16)
        # fp8 halo: rows [g0-HLO, g0) and [g0+512, g0+512+HHI)
        x8_a = np.zeros((HLO, 2 * R), np.float32)
        x8_a[:, :R] = Xp[g0:g0 + HLO]                      # lower halo
        x8_a[:HHI, R:] = Xp[HLO + g0 + NOUT * 128:
                            HLO + g0 + NOUT * 128 + HHI]   # upper halo
        # fp16 weight blocks
        wts_a = np.zeros((128, len(WBLOCKS) * 128), np.float32)
        for bi, (j, d) in enumerate(WBLOCKS):
            i0 = g0 + (j + d - 1) * 128
            o0 = g0 + j * 128
            wts_a[:, bi * 128:(bi + 1) * 128] = \
                Ap[HLO + i0:HLO + i0 + 128, o0:o0 + 128]
        # fp8 halo weight blocks
        w8_a = np.zeros((HLO, 2 * 128), np.float32)
        w8_a[:, :128] = Ap[g0:g0 + HLO, g0:g0 + 128]
        w8_a[:HHI, 128:] = Ap[HLO + g0 + NOUT * 128:
                              HLO + g0 + NOUT * 128 + HHI,
                              g0 + (NOUT - 1) * 128:g0 + NOUT * 128]
        # halo support must fit (A is banded; verified against the real A)
        assert np.all(Ap[:g0, g0:g0 + NOUT * 128] == 0.0)
        assert np.all(Ap[HLO + g0 + NOUT * 128 + HHI:,
                         g0:g0 + NOUT * 128] == 0.0)
        in_maps.append({
            "xin": np.ascontiguousarray(xin_a),
            "x8in": np.ascontiguousarray(_to_f8(x8_a)),
            "wts": np.ascontiguousarray(wts_a.astype(np.float16)),
            "w8": np.ascontiguousarray(_to_f8(w8_a)),
        })

    # host residual: the 4 real out positions of lin-tile 32 (f-bin 1024)
    residual = A[RES_IN0:L, RES_LO:L].T @ X[RES_IN0:L]    # [4, R] fp32
    return in_maps, residual


def _gather_output(results, bias_img, residual):
    out_lin = np.zeros((LPAD, R), np.float32)
    for cid in range(NCORES):
        g0 = cid * NOUT * 128
        out_lin[g0:g0 + NOUT * 128] = results[cid]["out"].astype(np.float32)
    out_lin[RES_LO:L] = residual
    # [L, R] -> (B, C, T, F):  lin = f*4+c, r = b*T+t
    out = out_lin[:L].reshape(F, C, B, T).transpose(2, 1, 3, 0)
    out = np.ascontiguousarray(out) + bias_img[None, :, None, :]
    return out.astype(np.float32)


def _run_on_device(in_maps, loop_iters=1, unroll=1):
    from concourse.bass_utils import run_bass_kernel_spmd
    nc = _build_program(loop_iters, unroll)
    res = run_bass_kernel_spmd(nc, in_maps, list(range(NCORES)))
    return res.results


def kernel(x, pre_weight, pre_bias, post_weight, post_bias, mask, ola_window,
           f_idxes):
    x = np.asarray(x, np.float32)
    pre_weight = np.asarray(pre_weight, np.float32)
    pre_bias = np.asarray(pre_bias, np.float32)
    post_weight = np.asarray(post_weight, np.float32)
    post_bias = np.asarray(post_bias, np.float32)
    mask = np.asarray(mask, np.float32)
    ola_window = np.asarray(ola_window, np.float32)
    f_idxes = np.asarray(f_idxes)

    A, bias_img = _build_A(pre_weight, pre_bias, post_weight, post_bias,
                           mask, ola_window, f_idxes)
    in_maps, residual = _shard_inputs(x, A)
    results = _run_on_device(in_maps)
    return _gather_output(results, bias_img, residual)


# revision 48
# speedup vs baseline: 1.1169x; 1.0062x over previous
"""Trainium2 Bass kernel for nn_BandSplit (grouped band einsum as banded matmul).

The reference computes, per (b, t) row:
    g = gather(x, f_idxes) * mask            # per-band slice of the spectrum
    h = einsum('ki,kio->ko', g, pre_weight) + pre_bias
    y = einsum('ko,koj->kj', h, post_weight) + post_bias
    out = scatter_add(y * mask) / ola_window

Because each band's nonzero bins are a contiguous f-range, the whole pipeline
is linear in x and collapses to ONE banded matrix multiply in the interleaved
linear space  lin = f*4 + c  (bandwidth <= 131 < 132):

    out_lin[l', r] = sum_l A[l, l'] * x_lin[l, r]
    A = sum_k scatter(diag(mask_k) @ W1_k @ W2_k @ diag(mask_k / ola))

A is built on the host from the (small) weight inputs.  x is pre-transposed on
the host into [lin, rows] tiles so the device does only contiguous DMA plus
dense 128x128x512 matmuls (fp32 PSUM accumulation) on 3 block-diagonals.

Sharding: 8 lin-groups of 4 out-tiles (of 128) x full rows, one per core.
Each out-tile j contracts its own tile (d=1) and both neighbours (d=0/d=2).
The group-edge halo paths carry only ~5% of the output energy, so they run
in fp8 (E4M3) at their true support width (<=112 below / <=104 above the
group), saving ~45% of the halo DMA bytes; everything else is fp16 with
fp32 PSUM.  Measured end-to-end error ~8e-3 vs the fp32 reference.

DMA layout is chosen for few, large transfers (the SP sequencer pays ~650ns
per issued DMA): one DMA per x tile / weight blob / out tile.  Matmuls are
ordered (out-tile, diagonal, chunk) so the PE keeps one stationary weight
block for 4 consecutive matmuls.
"""

import numpy as np

# ---- problem constants (hardcoded; harness supplies matching inputs) ----
B, C, T, F = 4, 4, 512, 1025
KB, WMAX = 256, 33
L = F * C                 # 4100 linear positions
NT = (L + 127) // 128     # 33 tiles of 128
LPAD = NT * 128           # 4224
R = B * T                 # 2048 rows (b, t)
NCORES = 8
ND = 3                    # block diagonals
CHUNK = 512               # PSUM bank (fp32) free-dim limit

# The last lin-tile (32) covers only 4 real positions (f-bin 1024); its
# output is computed on the host, so the device grid is exactly 32 tiles.
NT_DEV = 32
RES_LO = NT_DEV * 128            # 4096: first host-residual out position
RES_IN0 = RES_LO - (WMAX - 1) * C - C + 1  # input support start (3965)

NOUT = NT_DEV // NCORES   # 4 out-tiles per core
RC = R                    # rows per core (no row split)
NCHUNK = RC // CHUNK

# fp8 halo: true band support beyond a 512-lin group is <= 96 below and
# <= 92 above (asserted on the host against A).  Both halves AND their two
# fp8 weight blocks are packed in one [HLO, 2*RC + 256] fp8 tile: cols
# [0,RC) = lower halo, [RC,2RC) = upper halo, [2RC,2RC+128) = lower weight
# block, [2RC+128,2RC+256) = upper weight block (one DMA instead of two —
# each issued DMA costs ~0.5us minimum on HW).
HLO = 96
HHI = 92

# fp16 weight blocks, in (out-tile j, diagonal d) issue order; the two halo
# blocks (0,0) and (NOUT-1,2) live in the fp8 blob instead.
WBLOCKS = [(j, d) for j in range(NOUT) for d in range(ND)
           if (j, d) not in ((0, 0), (NOUT - 1, 2))]

_prog_cache = {}


def _build_program(loop_iters=1, unroll=1):
    """Uniform SPMD program: per core, NOUT out-tiles x 3 diagonals of
    [128,128] matmuls over [*,512] row chunks; halo diagonals in fp8.

    loop_iters > 1 wraps the body in a For_i replay loop (timing vehicle).
    `unroll` emits that many bodies per For_i iteration so the loop's
    all-engine back-edge barrier amortizes; loop_iters % unroll == 0."""
    import concourse.bacc as bacc
    import concourse.tile as tile
    import concourse.mybir as mybir

    key = (loop_iters, unroll)
    if key in _prog_cache:
        return _prog_cache[key]

    f32 = mybir.dt.float32
    f16 = mybir.dt.float16
    f8 = mybir.dt.float8e4

    nc = bacc.Bacc("TRN2", target_bir_lowering=False, debug=False,
                   num_devices=NCORES)
    xin = nc.dram_tensor("xin", [NOUT * 128, RC], f16,
                         kind="ExternalInput").ap()
    x8in = nc.dram_tensor("x8in", [HLO, 2 * RC + 2 * 128], f8,
                          kind="ExternalInput").ap()
    wts = nc.dram_tensor("wts", [128, len(WBLOCKS) * 128], f16,
                         kind="ExternalInput").ap()
    out = nc.dram_tensor("out", [NOUT * 128, RC], f16,
                         kind="ExternalOutput").ap()

    with tile.TileContext(nc) as tc:
        with (
            tc.tile_pool(name="xp", bufs=2) as xp,
            tc.tile_pool(name="wp", bufs=2) as wp,
            tc.tile_pool(name="yp", bufs=4) as yp,
            tc.tile_pool(name="pp", bufs=2, space="PSUM") as pp,
        ):
            def dma_in():
                # DMA issue order feeds the compute stream: out-tile 0 only
                # needs wt+x0+x1(+halo), so it can start while x2/x3 stream.
                wt = wp.tile([128, len(WBLOCKS) * 128], f16, tag="w",
                             name="wt")
                nc.sync.dma_start(wt[:], wts)
                xs = []
                for i in range(NOUT):
                    t = xp.tile([128, RC], f16, tag=f"x{i}", name=f"x{i}")
                    xs.append(t)
                for i in (0, 1):
                    nc.sync.dma_start(xs[i][:], xin[i * 128:(i + 1) * 128, :])
                x8t = xp.tile([HLO, 2 * RC + 2 * 128], f8, tag="x8",
                              name="x8t")
                nc.sync.dma_start(x8t[:], x8in)
                for i in range(2, NOUT):
                    nc.sync.dma_start(xs[i][:], xin[i * 128:(i + 1) * 128, :])
                return wt, xs, x8t

            def compute(handles):
                wt, xs, x8t = handles
                # (j, d) blocks in data-arrival order: each block is 4
                # same-stationary matmuls; two out-tiles accumulate in
                # flight (8 PSUM banks), so the PE never waits for x DMA.
                sched = [(0, 1), (1, 0), (0, 2), (1, 1), (0, 0), (1, 2),
                         (2, 0), (2, 1), (3, 0), (2, 2), (3, 1), (3, 2)]
                first = {j: min(i for i, (jj, _) in enumerate(sched) if jj == j)
                         for j in range(NOUT)}
                last = {j: max(i for i, (jj, _) in enumerate(sched) if jj == j)
                        for j in range(NOUT)}
                pss = {}
                for si, (j, d) in enumerate(sched):
                    if si == first[j]:
                        pss[j] = [pp.tile([128, CHUNK], f32, tag=f"ps{ch}",
                                          name=f"ps{ch}")
                                  for ch in range(NCHUNK)]
                    if (j, d) == (0, 0):
                        lhsT = x8t[0:HLO, 2 * RC:2 * RC + 128]
                    elif (j, d) == (NOUT - 1, 2):
                        lhsT = x8t[0:HHI, 2 * RC + 128:2 * RC + 256]
                    else:
                        bi = WBLOCKS.index((j, d))
                        lhsT = wt[:, bi * 128:(bi + 1) * 128]
                    for ch in range(NCHUNK):
                        c0 = ch * CHUNK
                        if (j, d) == (0, 0):
                            rhs = x8t[0:HLO, c0:c0 + CHUNK]
                        elif (j, d) == (NOUT - 1, 2):
                            rhs = x8t[0:HHI, RC + c0:RC + c0 + CHUNK]
                        else:
                            rhs = xs[j + d - 1][:, c0:c0 + CHUNK]
                        nc.tensor.matmul(pss[j][ch][:], lhsT, rhs,
                                         start=(si == first[j]),
                                         stop=(si == last[j]))
                    if si == last[j]:
                        y = yp.tile([128, RC], f16, tag="y", name="y")
                        for ch in range(NCHUNK):
                            dst = y[:, ch * CHUNK:(ch + 1) * CHUNK]
                            if (j * NCHUNK + ch) % 2 == 0:
                                nc.scalar.copy(dst, pss[j][ch][:])
                            else:
                                nc.vector.tensor_copy(dst, pss[j][ch][:])
                        nc.sync.dma_start(out[j * 128:(j + 1) * 128, :], y[:])

            if loop_iters == 1:
                compute(dma_in())
            else:
                # U bodies per For_i iteration: amortizes the loop's
                # all-engine barrier; next body's input DMAs issue before
                # the current body's compute so transfers stay back-to-back.
                assert loop_iters % unroll == 0
                with tc.For_i(0, loop_iters // unroll, 1) as _i:
                    h = dma_in()
                    for _u in range(unroll):
                        nh = dma_in() if _u < unroll - 1 else None
                        compute(h)
                        h = nh

    nc.compile()
    _prog_cache[key] = nc
    return nc


def _build_A(pre_weight, pre_bias, post_weight, post_bias, mask, ola_window,
             f_idxes):
    """Host: banded operator A[in_lin, out_lin] (LPAD x LPAD, fp32) and the
    constant bias image (C, F)."""
    fi = f_idxes.reshape(KB, WMAX).astype(np.int64)
    mk = mask.reshape(KB, WMAX).astype(np.float32)
    ola = ola_window.astype(np.float32)

    # effective per-band operators with mask and 1/ola folded in
    # row (input) index i = w*C + c ; col (output) index j = w'*C + c'
    mrow = np.repeat(mk, C, axis=1)                     # (KB, WMAX*C)
    inv_ola = np.where(ola != 0, 1.0 / ola, 0.0)
    ola_cols = inv_ola[fi]                              # (KB, WMAX)
    mcol = np.repeat(mk * ola_cols, C, axis=1)          # (KB, WMAX*C)

    w1 = pre_weight * mrow[:, :, None]                  # (KB, D, 128)
    w2 = post_weight * mcol[:, None, :]                 # (KB, 128, D)
    Mk = np.matmul(w1, w2)                              # (KB, D, D) fp32

    A = np.zeros((LPAD, LPAD), np.float32)
    lin = (fi[:, :, None] * C + np.arange(C)[None, None, :]).reshape(KB, -1)
    for k in range(KB):
        idx = lin[k]
        A[np.ix_(idx, idx)] += Mk[k]   # duplicate idx entries are all-zero rows/cols

    # bias: (pre_bias @ W2_raw + post_bias) * mask / ola, scattered -> (C, F)
    by = (np.einsum('ko,koj->kj', pre_bias, post_weight) + post_bias)  # (KB, D)
    by = by * mcol                                                      # masked + /ola
    bias_img = np.zeros((C, F), np.float32)
    np.add.at(bias_img,
              (np.tile(np.arange(C), (KB, WMAX, 1)).reshape(KB, -1),
               np.repeat(fi, C, axis=1)),
              by)
    return A, bias_img


def _to_f8(a):
    """TRN FP8_EXP4 (E4M3 with +-240 max) == ml_dtypes.float8_e4m3."""
    import ml_dtypes
    return np.clip(a, -240.0, 240.0).astype(ml_dtypes.float8_e4m3)


def _shard_inputs(x, A):
    """Per-core {xin, x8in, wts, w8} arrays + the host residual rows."""
    # x (B, C, T, F) -> X_lin [L, R], lin = f*4+c, r = b*T+t
    X = np.ascontiguousarray(
        x.transpose(3, 1, 0, 2).reshape(L, R).astype(np.float32))
    # halo-padded copies: HLO zero rows on top, zeros to lin 4096+HHI below
    Xp = np.zeros((HLO + RES_LO + HHI, R), np.float32)
    Xp[HLO:HLO + L] = X
    Ap = np.zeros((HLO + RES_LO + HHI, LPAD), np.float32)
    Ap[HLO:HLO + L] = A[:L]

    in_maps = []
    for cid in range(NCORES):
        g0 = cid * NOUT * 128            # first owned out lin
        # owned fp16 tiles
        xin_a = X[g0:g0 + NOUT * 128].astype(np.float16)
        # fp8 halo rows [g0-HLO, g0) and [g0+512, g0+512+HHI), plus the two
        # fp8 halo weight blocks, in one blob
        x8_a = np.zeros((HLO, 2 * R + 2 * 128), np.float32)
        x8_a[:, :R] = Xp[g0:g0 + HLO]                      # lower halo
        x8_a[:HHI, R:2 * R] = Xp[HLO + g0 + NOUT * 128:
                                 HLO + g0 + NOUT * 128 + HHI]  # upper halo
        x8_a[:, 2 * R:2 * R + 128] = Ap[g0:g0 + HLO, g0:g0 + 128]
        x8_a[:HHI, 2 * R + 128:] = Ap[HLO + g0 + NOUT * 128:
                                      HLO + g0 + NOUT * 128 + HHI,
                                      g0 + (NOUT - 1) * 128:g0 + NOUT * 128]
        # fp16 weight blocks
        wts_a = np.zeros((128, len(WBLOCKS) * 128), np.float32)
        for bi, (j, d) in enumerate(WBLOCKS):
            i0 = g0 + (j + d - 1) * 128
            o0 = g0 + j * 128
            wts_a[:, bi * 128:(bi + 1) * 128] = \
                Ap[HLO + i0:HLO + i0 + 128, o0:o0 + 128]
        # halo support must fit (A is banded; verified against the real A)
        assert np.all(Ap[:g0, g0:g0 + NOUT * 128] == 0.0)
        assert np.all(Ap[HLO + g0 + NOUT * 128 + HHI:,
                         g0:g0 + NOUT * 128] == 0.0)
        in_maps.append({
            "xin": np.ascontiguousarray(xin_a),
            "x8in": np.ascontiguousarray(_to_f8(x8_a)),
            "wts": np.ascontiguousarray(wts_a.astype(np.float16)),
        })

    # host residual: the 4 real out positions of lin-tile 32 (f-bin 1024)
    residual = A[RES_IN0:L, RES_LO:L].T @ X[RES_IN0:L]    # [4, R] fp32
    return in_maps, residual


def _gather_output(results, bias_img, residual):
    out_lin = np.zeros((LPAD, R), np.float32)
    for cid in range(NCORES):
        g0 = cid * NOUT * 128
        out_lin[g0:g0 + NOUT * 128] = results[cid]["out"].astype(np.float32)
    out_lin[RES_LO:L] = residual
    # [L, R] -> (B, C, T, F):  lin = f*4+c, r = b*T+t
    out = out_lin[:L].reshape(F, C, B, T).transpose(2, 1, 3, 0)
    out = np.ascontiguousarray(out) + bias_img[None, :, None, :]
    return out.astype(np.float32)


def _run_on_device(in_maps, loop_iters=1, unroll=1):
    from concourse.bass_utils import run_bass_kernel_spmd
    nc = _build_program(loop_iters, unroll)
    res = run_bass_kernel_spmd(nc, in_maps, list(range(NCORES)))
    return res.results


def kernel(x, pre_weight, pre_bias, post_weight, post_bias, mask, ola_window,
           f_idxes):
    x = np.asarray(x, np.float32)
    pre_weight = np.asarray(pre_weight, np.float32)
    pre_bias = np.asarray(pre_bias, np.float32)
    post_weight = np.asarray(post_weight, np.float32)
    post_bias = np.asarray(post_bias, np.float32)
    mask = np.asarray(mask, np.float32)
    ola_window = np.asarray(ola_window, np.float32)
    f_idxes = np.asarray(f_idxes)

    A, bias_img = _build_A(pre_weight, pre_bias, post_weight, post_bias,
                           mask, ola_window, f_idxes)
    in_maps, residual = _shard_inputs(x, A)
    results = _run_on_device(in_maps)
    return _gather_output(results, bias_img, residual)


# revision 50
# speedup vs baseline: 1.1214x; 1.0041x over previous
"""Trainium2 Bass kernel for nn_BandSplit (grouped band einsum as banded matmul).

The reference computes, per (b, t) row:
    g = gather(x, f_idxes) * mask            # per-band slice of the spectrum
    h = einsum('ki,kio->ko', g, pre_weight) + pre_bias
    y = einsum('ko,koj->kj', h, post_weight) + post_bias
    out = scatter_add(y * mask) / ola_window

Because each band's nonzero bins are a contiguous f-range, the whole pipeline
is linear in x and collapses to ONE banded matrix multiply in the interleaved
linear space  lin = f*4 + c  (bandwidth <= 131 < 132):

    out_lin[l', r] = sum_l A[l, l'] * x_lin[l, r]
    A = sum_k scatter(diag(mask_k) @ W1_k @ W2_k @ diag(mask_k / ola))

A is built on the host from the (small) weight inputs.  x is pre-transposed on
the host into [lin, rows] tiles so the device does only contiguous DMA plus
dense 128x128x512 matmuls (fp32 PSUM accumulation) on 3 block-diagonals.

Sharding: 8 lin-groups of 4 out-tiles (of 128) x full rows, one per core.
Each out-tile j contracts its own tile (d=1) and both neighbours (d=0/d=2).
The group-edge halo paths carry only ~5% of the output energy, so they run
in fp8 (E4M3) at their true support width (<=112 below / <=104 above the
group), saving ~45% of the halo DMA bytes; everything else is fp16 with
fp32 PSUM.  Measured end-to-end error ~8e-3 vs the fp32 reference.

DMA layout is chosen for few, large transfers (the SP sequencer pays ~650ns
per issued DMA): one DMA per x tile / weight blob / out tile.  Matmuls are
ordered (out-tile, diagonal, chunk) so the PE keeps one stationary weight
block for 4 consecutive matmuls.
"""

import numpy as np

# ---- problem constants (hardcoded; harness supplies matching inputs) ----
B, C, T, F = 4, 4, 512, 1025
KB, WMAX = 256, 33
L = F * C                 # 4100 linear positions
NT = (L + 127) // 128     # 33 tiles of 128
LPAD = NT * 128           # 4224
R = B * T                 # 2048 rows (b, t)
NCORES = 8
ND = 3                    # block diagonals
CHUNK = 512               # PSUM bank (fp32) free-dim limit

# The last lin-tile (32) covers only 4 real positions (f-bin 1024); its
# output is computed on the host, so the device grid is exactly 32 tiles.
NT_DEV = 32
RES_LO = NT_DEV * 128            # 4096: first host-residual out position
RES_IN0 = RES_LO - (WMAX - 1) * C - C + 1  # input support start (3965)

NOUT = NT_DEV // NCORES   # 4 out-tiles per core
RC = R                    # rows per core (no row split)
NCHUNK = RC // CHUNK

# fp8 halo: true band support beyond a 512-lin group is <= 96 below and
# <= 92 above (asserted on the host against A).  Both halves AND their two
# fp8 weight blocks are packed in one [HLO, 2*RC + 256] fp8 tile: cols
# [0,RC) = lower halo, [RC,2RC) = upper halo, [2RC,2RC+128) = lower weight
# block, [2RC+128,2RC+256) = upper weight block (one DMA instead of two —
# each issued DMA costs ~0.5us minimum on HW).
HLO = 96
HHI = 92

# fp16 weight blocks, in (out-tile j, diagonal d) issue order; the two halo
# blocks (0,0) and (NOUT-1,2) live in the fp8 blob instead.
WBLOCKS = [(j, d) for j in range(NOUT) for d in range(ND)
           if (j, d) not in ((0, 0), (NOUT - 1, 2))]

_prog_cache = {}


def _build_program(loop_iters=1, unroll=1):
    """Uniform SPMD program: per core, NOUT out-tiles x 3 diagonals of
    [128,128] matmuls over [*,512] row chunks; halo diagonals in fp8.

    loop_iters > 1 wraps the body in a For_i replay loop (timing vehicle).
    `unroll` emits that many bodies per For_i iteration so the loop's
    all-engine back-edge barrier amortizes; loop_iters % unroll == 0."""
    import concourse.bacc as bacc
    import concourse.tile as tile
    import concourse.mybir as mybir

    key = (loop_iters, unroll)
    if key in _prog_cache:
        return _prog_cache[key]

    f32 = mybir.dt.float32
    f16 = mybir.dt.float16
    f8 = mybir.dt.float8e4

    nc = bacc.Bacc("TRN2", target_bir_lowering=False, debug=False,
                   num_devices=NCORES)
    xin = nc.dram_tensor("xin", [NOUT * 128, RC], f16,
                         kind="ExternalInput").ap()
    x8in = nc.dram_tensor("x8in", [HLO, 2 * RC + 2 * 128], f8,
                          kind="ExternalInput").ap()
    wts = nc.dram_tensor("wts", [128, len(WBLOCKS) * 128], f16,
                         kind="ExternalInput").ap()
    out = nc.dram_tensor("out", [NOUT * 128, RC], f16,
                         kind="ExternalOutput").ap()

    with tile.TileContext(nc) as tc:
        with (
            tc.tile_pool(name="xp", bufs=2) as xp,
            tc.tile_pool(name="wp", bufs=2) as wp,
            tc.tile_pool(name="yp", bufs=4) as yp,
            tc.tile_pool(name="pp", bufs=2, space="PSUM") as pp,
        ):
            def dma_in():
                # DMA issue order feeds the compute stream: out-tile 0 only
                # needs wt+x0+x1(+halo), so it can start while x2/x3 stream.
                wt = wp.tile([128, len(WBLOCKS) * 128], f16, tag="w",
                             name="wt")
                nc.sync.dma_start(wt[:], wts)
                xs = []
                for i in range(NOUT):
                    t = xp.tile([128, RC], f16, tag=f"x{i}", name=f"x{i}")
                    xs.append(t)
                for i in (0, 1):
                    nc.sync.dma_start(xs[i][:], xin[i * 128:(i + 1) * 128, :])
                x8t = xp.tile([HLO, 2 * RC + 2 * 128], f8, tag="x8",
                              name="x8t")
                nc.sync.dma_start(x8t[:], x8in)
                for i in range(2, NOUT):
                    nc.sync.dma_start(xs[i][:], xin[i * 128:(i + 1) * 128, :])
                return wt, xs, x8t

            def compute(handles):
                wt, xs, x8t = handles
                # (j, d) blocks in data-arrival order: each block is 4
                # same-stationary matmuls; two out-tiles accumulate in
                # flight (8 PSUM banks), so the PE never waits for x DMA.
                sched = [(0, 1), (1, 0), (0, 2), (1, 1), (0, 0), (1, 2),
                         (2, 0), (2, 1), (3, 0), (2, 2), (3, 1), (3, 2)]
                first = {j: min(i for i, (jj, _) in enumerate(sched) if jj == j)
                         for j in range(NOUT)}
                last = {j: max(i for i, (jj, _) in enumerate(sched) if jj == j)
                        for j in range(NOUT)}
                pss = {}
                for si, (j, d) in enumerate(sched):
                    if si == first[j]:
                        pss[j] = [pp.tile([128, CHUNK], f32, tag=f"ps{ch}",
                                          name=f"ps{ch}")
                                  for ch in range(NCHUNK)]
                    if (j, d) == (0, 0):
                        lhsT = x8t[0:HLO, 2 * RC:2 * RC + 128]
                    elif (j, d) == (NOUT - 1, 2):
                        lhsT = x8t[0:HHI, 2 * RC + 128:2 * RC + 256]
                    else:
                        bi = WBLOCKS.index((j, d))
                        lhsT = wt[:, bi * 128:(bi + 1) * 128]
                    for ch in range(NCHUNK):
                        c0 = ch * CHUNK
                        if (j, d) == (0, 0):
                            rhs = x8t[0:HLO, c0:c0 + CHUNK]
                        elif (j, d) == (NOUT - 1, 2):
                            rhs = x8t[0:HHI, RC + c0:RC + c0 + CHUNK]
                        else:
                            rhs = xs[j + d - 1][:, c0:c0 + CHUNK]
                        nc.tensor.matmul(pss[j][ch][:], lhsT, rhs,
                                         start=(si == first[j]),
                                         stop=(si == last[j]))
                    if si == last[j]:
                        y = yp.tile([128, RC], f16, tag="y", name="y")
                        for ch in range(NCHUNK):
                            dst = y[:, ch * CHUNK:(ch + 1) * CHUNK]
                            if (j * NCHUNK + ch) % 2 == 0:
                                nc.scalar.copy(dst, pss[j][ch][:])
                            else:
                                nc.vector.tensor_copy(dst, pss[j][ch][:])
                        # issue out-DMAs from the Activation engine's own
                        # HWDGE path so the output stream can overlap the
                        # SP-issued input stream on separate queues
                        nc.scalar.dma_start(out[j * 128:(j + 1) * 128, :],
                                            y[:])

            if loop_iters == 1:
                compute(dma_in())
            else:
                # U bodies per For_i iteration: amortizes the loop's
                # all-engine barrier; next body's input DMAs issue before
                # the current body's compute so transfers stay back-to-back.
                assert loop_iters % unroll == 0
                with tc.For_i(0, loop_iters // unroll, 1) as _i:
                    h = dma_in()
                    for _u in range(unroll):
                        nh = dma_in() if _u < unroll - 1 else None
                        compute(h)
                        h = nh

    nc.compile()
    _prog_cache[key] = nc
    return nc


def _build_A(pre_weight, pre_bias, post_weight, post_bias, mask, ola_window,
             f_idxes):
    """Host: banded operator A[in_lin, out_lin] (LPAD x LPAD, fp32) and the
    constant bias image (C, F)."""
    fi = f_idxes.reshape(KB, WMAX).astype(np.int64)
    mk = mask.reshape(KB, WMAX).astype(np.float32)
    ola = ola_window.astype(np.float32)

    # effective per-band operators with mask and 1/ola folded in
    # row (input) index i = w*C + c ; col (output) index j = w'*C + c'
    mrow = np.repeat(mk, C, axis=1)                     # (KB, WMAX*C)
    inv_ola = np.where(ola != 0, 1.0 / ola, 0.0)
    ola_cols = inv_ola[fi]                              # (KB, WMAX)
    mcol = np.repeat(mk * ola_cols, C, axis=1)          # (KB, WMAX*C)

    w1 = pre_weight * mrow[:, :, None]                  # (KB, D, 128)
    w2 = post_weight * mcol[:, None, :]                 # (KB, 128, D)
    Mk = np.matmul(w1, w2)                              # (KB, D, D) fp32

    A = np.zeros((LPAD, LPAD), np.float32)
    lin = (fi[:, :, None] * C + np.arange(C)[None, None, :]).reshape(KB, -1)
    for k in range(KB):
        idx = lin[k]
        A[np.ix_(idx, idx)] += Mk[k]   # duplicate idx entries are all-zero rows/cols

    # bias: (pre_bias @ W2_raw + post_bias) * mask / ola, scattered -> (C, F)
    by = (np.einsum('ko,koj->kj', pre_bias, post_weight) + post_bias)  # (KB, D)
    by = by * mcol                                                      # masked + /ola
    bias_img = np.zeros((C, F), np.float32)
    np.add.at(bias_img,
              (np.tile(np.arange(C), (KB, WMAX, 1)).reshape(KB, -1),
               np.repeat(fi, C, axis=1)),
              by)
    return A, bias_img


def _to_f8(a):
    """TRN FP8_EXP4 (E4M3 with +-240 max) == ml_dtypes.float8_e4m3."""
    import ml_dtypes
    return np.clip(a, -240.0, 240.0).astype(ml_dtypes.float8_e4m3)


def _shard_inputs(x, A):
    """Per-core {xin, x8in, wts, w8} arrays + the host residual rows."""
    # x (B, C, T, F) -> X_lin [L, R], lin = f*4+c, r = b*T+t
    X = np.ascontiguousarray(
        x.transpose(3, 1, 0, 2).reshape(L, R).astype(np.float32))
    # halo-padded copies: HLO zero rows on top, zeros to lin 4096+HHI below
    Xp = np.zeros((HLO + RES_LO + HHI, R), np.float32)
    Xp[HLO:HLO + L] = X
    Ap = np.zeros((HLO + RES_LO + HHI, LPAD), np.float32)
    Ap[HLO:HLO + L] = A[:L]

    in_maps = []
    for cid in range(NCORES):
        g0 = cid * NOUT * 128            # first owned out lin
        # owned fp16 tiles
        xin_a = X[g0:g0 + NOUT * 128].astype(np.float16)
        # fp8 halo rows [g0-HLO, g0) and [g0+512, g0+512+HHI), plus the two
        # fp8 halo weight blocks, in one blob
        x8_a = np.zeros((HLO, 2 * R + 2 * 128), np.float32)
        x8_a[:, :R] = Xp[g0:g0 + HLO]                      # lower halo
        x8_a[:HHI, R:2 * R] = Xp[HLO + g0 + NOUT * 128:
                                 HLO + g0 + NOUT * 128 + HHI]  # upper halo
        x8_a[:, 2 * R:2 * R + 128] = Ap[g0:g0 + HLO, g0:g0 + 128]
        x8_a[:HHI, 2 * R + 128:] = Ap[HLO + g0 + NOUT * 128:
                                      HLO + g0 + NOUT * 128 + HHI,
                                      g0 + (NOUT - 1) * 128:g0 + NOUT * 128]
        # fp16 weight blocks
        wts_a = np.zeros((128, len(WBLOCKS) * 128), np.float32)
        for bi, (j, d) in enumerate(WBLOCKS):
            i0 = g0 + (j + d - 1) * 128
            o0 = g0 + j * 128
            wts_a[:, bi * 128:(bi + 1) * 128] = \
                Ap[HLO + i0:HLO + i0 + 128, o0:o0 + 128]
        # halo support must fit (A is banded; verified against the real A)
        assert np.all(Ap[:g0, g0:g0 + NOUT * 128] == 0.0)
        assert np.all(Ap[HLO + g0 + NOUT * 128 + HHI:,
                         g0:g0 + NOUT * 128] == 0.0)
        in_maps.append({
            "xin": np.ascontiguousarray(xin_a),
            "x8in": np.ascontiguousarray(_to_f8(x8_a)),
            "wts": np.ascontiguousarray(wts_a.astype(np.float16)),
        })

    # host residual: the 4 real out positions of lin-tile 32 (f-bin 1024)
    residual = A[RES_IN0:L, RES_LO:L].T @ X[RES_IN0:L]    # [4, R] fp32
    return in_maps, residual


def _gather_output(results, bias_img, residual):
    out_lin = np.zeros((LPAD, R), np.float32)
    for cid in range(NCORES):
        g0 = cid * NOUT * 128
        out_lin[g0:g0 + NOUT * 128] = results[cid]["out"].astype(np.float32)
    out_lin[RES_LO:L] = residual
    # [L, R] -> (B, C, T, F):  lin = f*4+c, r = b*T+t
    out = out_lin[:L].reshape(F, C, B, T).transpose(2, 1, 3, 0)
    out = np.ascontiguousarray(out) + bias_img[None, :, None, :]
    return out.astype(np.float32)


def _run_on_device(in_maps, loop_iters=1, unroll=1):
    from concourse.bass_utils import run_bass_kernel_spmd
    nc = _build_program(loop_iters, unroll)
    res = run_bass_kernel_spmd(nc, in_maps, list(range(NCORES)))
    return res.results


def kernel(x, pre_weight, pre_bias, post_weight, post_bias, mask, ola_window,
           f_idxes):
    x = np.asarray(x, np.float32)
    pre_weight = np.asarray(pre_weight, np.float32)
    pre_bias = np.asarray(pre_bias, np.float32)
    post_weight = np.asarray(post_weight, np.float32)
    post_bias = np.asarray(post_bias, np.float32)
    mask = np.asarray(mask, np.float32)
    ola_window = np.asarray(ola_window, np.float32)
    f_idxes = np.asarray(f_idxes)

    A, bias_img = _build_A(pre_weight, pre_bias, post_weight, post_bias,
                           mask, ola_window, f_idxes)
    in_maps, residual = _shard_inputs(x, A)
    results = _run_on_device(in_maps)
    return _gather_output(results, bias_img, residual)
